# revision 18
# baseline (speedup 1.0000x reference)
"""Trainium2 Bass kernel for DipolePredictorE3NN.

Reference computation (per batch b of B=1024, over N=4096 nodes):
    s      = feats[..., :5] @ w_path0                      # scalar gate
    tp     = C01 * s * edge + C11*w_path1[0] * cross(feats[...,5:8], edge)
    g      = tp.mean(nodes)                                # [B, 3]
    out    = relu(g @ W1 + b1) @ W2 + b2                   # [B, 3]

Strategy: data-parallel over batch, 8 cores x 128 batches; partition dim
= local batch (exactly 128), free dim = nodes.

Measured constraints that shaped the design:
  - strided SBUF reads run at 2-4 cyc/elem on DVE/ACT (vs 1 dense), so
    the host pre-arranges each core's shard tile-major channel-planar:
    for each node-tile, [128, C*T] with per-partition contiguous chunks
    (dense unit-stride planes on chip, 128 fat DMA descriptors).
  - GpSimd running concurrently with DVE poisons both (~2.6x), so Pool
    does no hot-loop work at all.
  - DMA does not interfere with DVE; HBM streaming (~23MB @ ~380GB/s =
    ~60us) overlaps compute.
  - DVE is the floor: 13 ops/tile-elem (9 fused multiply-reduce + 4
    gate adds) ~= 1 cyc/elem each.
w_path0/w_path1 are baked as immediates with path constants pre-folded.
Per node-tile:
  - ScalarE: tmp_u = (C01/N * w0_u) * x_u   (5 dense muls)
  - DVE: 4 tensor_adds build s; 9 affine_mul_reduce (custom DVE op,
    out=(in0*scale)*in1, accum_out=sum) write per-(tile,term) partial
    columns; cross terms use scale=+-C11*w1/N, cross AMRs issue first
    so they overlap the s build.
Partials fold with one strided tensor_reduce per k. g [128, 0:3] plus a
ones column feeds a PE transpose, then the MLP runs on the PE in
transposed form (b1 folded into the contraction, b2 via the activation
bias): hT = relu(W1b^T . [g|1]T), outT = W2^T . hT + b2. Per-core
output outT [3, 128]; the host concatenates and transposes.
"""

import sys

if "/opt/trn_rl_repo" not in sys.path:
    sys.path.insert(0, "/opt/trn_rl_repo")

import numpy as np

C01 = float(np.sqrt(0.5) / np.sqrt(3.0))
C11 = float(np.sqrt(0.5) / np.sqrt(6.0))

B, N = 1024, 4096
NCORES = 8
BL = B // NCORES  # 128 local batches = partition count

# small edge tiles: fast pipeline ramp, short critical tail
TILES = [256, 768, 1024, 1024, 768, 256]
assert sum(TILES) == N
TMAX = max(TILES)

_CACHED = {}


def _build(w0_vals, w1_val):
    import concourse.bacc as bacc
    import concourse.mybir as mybir
    from concourse import tile
    from concourse.masks import make_identity

    f32 = mybir.dt.float32
    Alu = mybir.AluOpType
    Act = mybir.ActivationFunctionType

    w0s = [float(w) * C01 / float(N) for w in w0_vals]  # pre-scaled gate weights
    c2 = float(w1_val) * C11 / float(N)  # cross-product coefficient

    nc = bacc.Bacc("TRN2", debug=False)

    feats = nc.dram_tensor("feats", [BL, 8 * N], f32, kind="ExternalInput").ap()
    edge = nc.dram_tensor("edge", [BL, 3 * N], f32, kind="ExternalInput").ap()
    W1 = nc.dram_tensor("W1", [3, 128], f32, kind="ExternalInput").ap()
    b1 = nc.dram_tensor("b1", [1, 128], f32, kind="ExternalInput").ap()
    W2 = nc.dram_tensor("W2", [128, 3], f32, kind="ExternalInput").ap()
    b2 = nc.dram_tensor("b2", [3, 1], f32, kind="ExternalInput").ap()
    outT = nc.dram_tensor("outT", [3, BL], f32, kind="ExternalOutput").ap()

    # cross product: (v x e)_k = v_a*e_b - v_b*e_a with (a,b) = (k+1, k+2) mod 3
    CROSS = [((k + 1) % 3, (k + 2) % 3) for k in range(3)]
    NT_ = len(TILES)

    with tile.TileContext(nc) as tc:
        with (
            tc.tile_pool(name="consts", bufs=1) as consts,
            tc.tile_pool(name="state", bufs=1) as state,
            tc.tile_pool(name="fio", bufs=3) as fio,
            tc.tile_pool(name="eio", bufs=4) as eio,
            tc.tile_pool(name="sw", bufs=2) as sw,
            tc.tile_pool(name="psum", bufs=1, space="PSUM") as psum,
        ):
            # first tile's streams before anything else: the ramp is
            # gated by these transfers. Feats split into the vector part
            # (ch 5-7, feeds the cross AMRs immediately) and the scalar
            # part (ch 0-4, feeds the gate muls).
            Tt0 = TILES[0]
            ftile0 = fio.tile([128, 8 * TMAX], f32, tag="f", name="ftile0")
            etile0 = eio.tile([128, 3 * TMAX], f32, tag="e", name="etile0")
            nc.sync.dma_start(
                out=ftile0[:, 5 * Tt0 : 8 * Tt0], in_=feats[:, 5 * Tt0 : 8 * Tt0]
            )
            nc.sync.dma_start(out=etile0[:, : 3 * Tt0], in_=edge[:, : 3 * Tt0])
            nc.sync.dma_start(
                out=ftile0[:, : 5 * Tt0], in_=feats[:, : 5 * Tt0]
            )

            # constants + Pool-side setup: Pool/PE must be quiet during
            # the DVE hot loop, so identity generation happens up front
            identity = consts.tile([128, 128], f32)
            make_identity(nc, identity[:])
            w1b_s = consts.tile([4, 128], f32)
            nc.sync.dma_start(out=w1b_s[0:3, :], in_=W1)
            nc.sync.dma_start(out=w1b_s[3:4, :], in_=b1)
            w2_s = consts.tile([128, 3], f32)
            nc.sync.dma_start(out=w2_s[:], in_=W2)
            b2_s = consts.tile([3, 1], f32)
            nc.sync.dma_start(out=b2_s[:], in_=b2)

            # acc[:, 0:3] holds g; col 3 = 1.0 feeds the bias fold
            acc = state.tile([128, 4], f32)
            nc.vector.memset(acc[:, 3:4], 1.0)

            # per-(tile, term) partial sums: col t*9 + 3k + {0: s*e_k,
            # 1: +cross, 2: -cross}; summed into acc at the end
            pcol = state.tile([128, NT_ * 9], f32)

            dummy = state.tile([128, TMAX], f32)

            foff = 0
            eoff = 0
            for t, Tt in enumerate(TILES):
                if t == 0:
                    ftile, etile = ftile0, etile0
                else:
                    ftile = fio.tile([128, 8 * TMAX], f32, tag="f", name="ftile")
                    etile = eio.tile([128, 3 * TMAX], f32, tag="e", name="etile")
                    nc.sync.dma_start(
                        out=ftile[:, 5 * Tt : 8 * Tt],
                        in_=feats[:, foff + 5 * Tt : foff + 8 * Tt],
                    )
                    nc.sync.dma_start(
                        out=etile[:, : 3 * Tt], in_=edge[:, eoff : eoff + 3 * Tt]
                    )
                    nc.sync.dma_start(
                        out=ftile[:, : 5 * Tt], in_=feats[:, foff : foff + 5 * Tt]
                    )
                foff += 8 * Tt
                eoff += 3 * Tt

                x = [ftile[:, u * Tt : (u + 1) * Tt] for u in range(8)]
                e = [etile[:, k * Tt : (k + 1) * Tt] for k in range(3)]

                # ScalarE: the 5 gate muls (independent; overlap the
                # cross AMRs below)
                s_buf = sw.tile([128, TMAX], f32, tag="s", name="s_buf")
                tmps = [
                    sw.tile([128, TMAX], f32, tag=f"tmp{u}", name=f"tmp{u}")
                    for u in range(1, 5)
                ]
                nc.scalar.mul(s_buf[:, :Tt], x[0], w0s[0])
                for u in range(1, 5):
                    nc.scalar.mul(tmps[u - 1][:, :Tt], x[u], w0s[u])

                # DVE: cross-product AMRs first (only need the DMA'd tiles)
                for k in range(3):
                    a, b = CROSS[k]
                    base = t * 9 + 3 * k
                    nc.vector.affine_mul_reduce(
                        out=dummy[:, :Tt], accum_out=pcol[:, base + 1 : base + 2],
                        in0=x[5 + a], in1=e[b], scale=c2, bias=0.0,
                    )
                    nc.vector.affine_mul_reduce(
                        out=dummy[:, :Tt], accum_out=pcol[:, base + 2 : base + 3],
                        in0=x[5 + b], in1=e[a], scale=-c2, bias=0.0,
                    )

                # DVE: fold the gate terms, then the 3 s*e_k AMRs
                for u in range(4):
                    nc.vector.tensor_add(
                        s_buf[:, :Tt], s_buf[:, :Tt], tmps[u][:, :Tt]
                    )
                for k in range(3):
                    base = t * 9 + 3 * k
                    nc.vector.affine_mul_reduce(
                        out=dummy[:, :Tt], accum_out=pcol[:, base : base + 1],
                        in0=s_buf[:, :Tt], in1=e[k], scale=1.0, bias=0.0,
                    )

            # --- fold partials: acc[:, k] = sum over tiles and terms ---
            pcol3 = pcol[:].rearrange("p (t j) -> p t j", j=9)
            for k in range(3):
                nc.vector.tensor_reduce(
                    out=acc[:, k : k + 1], in_=pcol3[:, :, 3 * k : 3 * k + 3],
                    axis=mybir.AxisListType.XY, op=Alu.add,
                )

            # --- gT = transpose([g|1]): [128, 4] -> [4, 128] via PE ---
            gT_ps = psum.tile([4, 128], f32)
            nc.tensor.transpose(gT_ps[:], acc[:], identity[:])
            gT = state.tile([4, 128], f32)
            nc.scalar.copy(gT[:], gT_ps[:])

            # --- hT = relu(W1b^T(k,m) contracted with gT(k,n)) ---
            h_ps = psum.tile([128, 128], f32)
            nc.tensor.matmul(h_ps[:], lhsT=w1b_s[:], rhs=gT[:], start=True, stop=True)
            hT = state.tile([128, 128], f32)
            nc.scalar.activation(hT[:], h_ps[:], Act.Relu)

            # --- outT = W2^T . hT + b2 ---
            o_ps = psum.tile([3, 128], f32)
            nc.tensor.matmul(o_ps[:], lhsT=w2_s[:], rhs=hT[:], start=True, stop=True)
            oT = state.tile([3, 128], f32)
            nc.scalar.activation(oT[:], o_ps[:], Act.Identity, bias=b2_s[:])
            nc.sync.dma_start(out=outT, in_=oT[:])

    nc.finalize()
    return nc


def _get_nc(w_path0, w_path1):
    key = (
        np.asarray(w_path0, np.float32).tobytes(),
        np.asarray(w_path1, np.float32).tobytes(),
    )
    if _CACHED.get("key") != key:
        _CACHED["nc"] = _build(
            np.asarray(w_path0, np.float32).reshape(5),
            float(np.asarray(w_path1, np.float32).reshape(1)[0]),
        )
        _CACHED["key"] = key
    return _CACHED["nc"]


def _tile_major(shard, C):
    """[BL, N, C] -> [BL, sum_t C*Tt]: per tile, channel-planar planes."""
    blocks = []
    off = 0
    for Tt in TILES:
        blk = shard[:, off : off + Tt, :].transpose(0, 2, 1).reshape(BL, C * Tt)
        blocks.append(blk)
        off += Tt
    return np.ascontiguousarray(np.concatenate(blocks, axis=1))


def _in_maps(feats, edge_attr, W1, b1, W2, b2):
    f32 = np.float32
    W1m = np.ascontiguousarray(W1, f32).reshape(3, 128)
    b1m = np.ascontiguousarray(b1, f32).reshape(1, 128)
    W2m = np.ascontiguousarray(W2, f32).reshape(128, 3)
    b2m = np.ascontiguousarray(b2, f32).reshape(3, 1)
    feats = np.asarray(feats, f32)
    edge_attr = np.asarray(edge_attr, f32)
    maps = []
    for c in range(NCORES):
        sl = slice(c * BL, (c + 1) * BL)
        maps.append(
            {
                "feats": _tile_major(feats[sl], 8),
                "edge": _tile_major(edge_attr[sl], 3),
                "W1": W1m,
                "b1": b1m,
                "W2": W2m,
                "b2": b2m,
            }
        )
    return maps


def run(inputs, trace=False, tmpdir=None):
    """Run on 8 cores; returns (out [B,3], BassKernelResults)."""
    from concourse import bass_utils

    nc = _get_nc(inputs["w_path0"], inputs["w_path1"])
    maps = _in_maps(
        inputs["feats"], inputs["edge_attr"],
        inputs["W1"], inputs["b1"], inputs["W2"], inputs["b2"],
    )
    kw = {}
    if trace:
        kw.update(trace=True, tmpdir=tmpdir)
    res = bass_utils.run_bass_kernel_spmd(
        nc, maps, core_ids=list(range(NCORES)), **kw
    )
    outT_full = np.concatenate([r["outT"] for r in res.results], axis=1)  # [3, B]
    return np.ascontiguousarray(outT_full.T), res


def kernel(feats, edge_attr, w_path0, w_path1, W1, b1, W2, b2):
    out, _ = run(
        dict(
            feats=feats, edge_attr=edge_attr, w_path0=w_path0, w_path1=w_path1,
            W1=W1, b1=b1, W2=W2, b2=b2,
        )
    )
    return out


# revision 22
# speedup vs baseline: 1.0250x; 1.0250x over previous
"""Trainium2 Bass kernel for DipolePredictorE3NN.

Reference computation (per batch b of B=1024, over N=4096 nodes):
    s      = feats[..., :5] @ w_path0                      # scalar gate
    tp     = C01 * s * edge + C11*w_path1[0] * cross(feats[...,5:8], edge)
    g      = tp.mean(nodes)                                # [B, 3]
    out    = relu(g @ W1 + b1) @ W2 + b2                   # [B, 3]

Strategy: data-parallel over batch, 8 cores x 128 batches; partition dim
= local batch (exactly 128), free dim = nodes.

Measured constraints that shaped the design:
  - strided SBUF reads run at 2-4 cyc/elem on DVE/ACT (vs 1 dense), so
    the host pre-arranges each core's shard tile-major channel-planar:
    for each node-tile, [128, C*T] with per-partition contiguous chunks
    (dense unit-stride planes on chip, 128 fat DMA descriptors).
  - GpSimd running concurrently with DVE poisons both (~2.6x), so Pool
    does no hot-loop work at all.
  - DMA does not interfere with DVE; HBM streaming (~23MB @ ~380GB/s =
    ~60us) overlaps compute.
  - DVE is the floor: 13 ops/tile-elem (9 fused multiply-reduce + 4
    gate adds) ~= 1 cyc/elem each.
w_path0/w_path1 are baked as immediates with path constants pre-folded.
Per node-tile:
  - ScalarE: tmp_u = (C01/N * w0_u) * x_u   (5 dense muls)
  - DVE: 4 tensor_adds build s; 9 affine_mul_reduce (custom DVE op,
    out=(in0*scale)*in1, accum_out=sum) write per-(tile,term) partial
    columns; cross terms use scale=+-C11*w1/N, cross AMRs issue first
    so they overlap the s build.
Partials fold with one strided tensor_reduce per k. g [128, 0:3] plus a
ones column feeds a PE transpose, then the MLP runs on the PE in
transposed form (b1 folded into the contraction, b2 via the activation
bias): hT = relu(W1b^T . [g|1]T), outT = W2^T . hT + b2. Per-core
output outT [3, 128]; the host concatenates and transposes.
"""

import sys

if "/opt/trn_rl_repo" not in sys.path:
    sys.path.insert(0, "/opt/trn_rl_repo")

import numpy as np

C01 = float(np.sqrt(0.5) / np.sqrt(3.0))
C11 = float(np.sqrt(0.5) / np.sqrt(6.0))

B, N = 1024, 4096
NCORES = 8
BL = B // NCORES  # 128 local batches = partition count

# small edge tiles: fast pipeline ramp, short critical tail
TILES = [256, 768, 1024, 1024, 768, 256]
assert sum(TILES) == N
TMAX = max(TILES)

_CACHED = {}


def _build(w0_vals, w1_val):
    import concourse.bacc as bacc
    import concourse.mybir as mybir
    from concourse import tile
    from concourse.masks import make_identity

    f32 = mybir.dt.float32
    Alu = mybir.AluOpType
    Act = mybir.ActivationFunctionType

    w0s = [float(w) * C01 / float(N) for w in w0_vals]  # pre-scaled gate weights
    c2 = float(w1_val) * C11 / float(N)  # cross-product coefficient

    nc = bacc.Bacc("TRN2", debug=False)

    feats = nc.dram_tensor("feats", [BL, 8 * N], f32, kind="ExternalInput").ap()
    edge = nc.dram_tensor("edge", [BL, 3 * N], f32, kind="ExternalInput").ap()
    W1 = nc.dram_tensor("W1", [3, 128], f32, kind="ExternalInput").ap()
    b1 = nc.dram_tensor("b1", [1, 128], f32, kind="ExternalInput").ap()
    W2 = nc.dram_tensor("W2", [128, 3], f32, kind="ExternalInput").ap()
    b2 = nc.dram_tensor("b2", [3, 1], f32, kind="ExternalInput").ap()
    outT = nc.dram_tensor("outT", [3, BL], f32, kind="ExternalOutput").ap()

    # cross product: (v x e)_k = v_a*e_b - v_b*e_a with (a,b) = (k+1, k+2) mod 3
    CROSS = [((k + 1) % 3, (k + 2) % 3) for k in range(3)]
    NT_ = len(TILES)

    with tile.TileContext(nc) as tc:
        with (
            tc.tile_pool(name="consts", bufs=1) as consts,
            tc.tile_pool(name="state", bufs=1) as state,
            tc.tile_pool(name="fio", bufs=3) as fio,
            tc.tile_pool(name="eio", bufs=4) as eio,
            tc.tile_pool(name="sw", bufs=2) as sw,
            tc.tile_pool(name="psum", bufs=1, space="PSUM") as psum,
        ):
            # first tile's streams before anything else: the ramp is
            # gated by these transfers. Feats split into the vector part
            # (ch 5-7, feeds the cross AMRs immediately) and the scalar
            # part (ch 0-4, feeds the gate muls).
            Tt0 = TILES[0]
            ftile0 = fio.tile([128, 8 * TMAX], f32, tag="f", name="ftile0")
            nc.sync.dma_start(out=ftile0[:, : 8 * Tt0], in_=feats[:, : 8 * Tt0])
            etile0 = eio.tile([128, 3 * TMAX], f32, tag="e", name="etile0")
            nc.sync.dma_start(out=etile0[:, : 3 * Tt0], in_=edge[:, : 3 * Tt0])

            # constants + Pool-side setup: Pool/PE must be quiet during
            # the DVE hot loop, so identity generation happens up front
            identity = consts.tile([128, 128], f32)
            make_identity(nc, identity[:])
            w1b_s = consts.tile([4, 128], f32)
            nc.sync.dma_start(out=w1b_s[0:3, :], in_=W1)
            nc.sync.dma_start(out=w1b_s[3:4, :], in_=b1)
            w2_s = consts.tile([128, 3], f32)
            nc.sync.dma_start(out=w2_s[:], in_=W2)
            b2_s = consts.tile([3, 1], f32)
            nc.sync.dma_start(out=b2_s[:], in_=b2)

            # acc[:, 0:3] holds g; col 3 = 1.0 feeds the bias fold
            acc = state.tile([128, 4], f32)
            nc.vector.memset(acc[:, 3:4], 1.0)

            # per-(tile, term) partial sums: col t*9 + 3k + {0: s*e_k,
            # 1: +cross, 2: -cross}; summed into acc at the end
            pcol = state.tile([128, NT_ * 9], f32)

            # stride-0 write target for the discarded AMR elementwise out
            dummy = state.tile([128, 1], f32)

            foff = 0
            eoff = 0
            for t, Tt in enumerate(TILES):
                if t == 0:
                    ftile, etile = ftile0, etile0
                else:
                    ftile = fio.tile([128, 8 * TMAX], f32, tag="f", name="ftile")
                    nc.sync.dma_start(
                        out=ftile[:, : 8 * Tt], in_=feats[:, foff : foff + 8 * Tt]
                    )
                    etile = eio.tile([128, 3 * TMAX], f32, tag="e", name="etile")
                    nc.sync.dma_start(
                        out=etile[:, : 3 * Tt], in_=edge[:, eoff : eoff + 3 * Tt]
                    )
                foff += 8 * Tt
                eoff += 3 * Tt

                x = [ftile[:, u * Tt : (u + 1) * Tt] for u in range(8)]
                e = [etile[:, k * Tt : (k + 1) * Tt] for k in range(3)]

                # ScalarE: the 5 gate muls (independent; overlap the
                # cross AMRs below)
                s_buf = sw.tile([128, TMAX], f32, tag="s", name="s_buf")
                tmps = [
                    sw.tile([128, TMAX], f32, tag=f"tmp{u}", name=f"tmp{u}")
                    for u in range(1, 5)
                ]
                nc.scalar.mul(s_buf[:, :Tt], x[0], w0s[0])
                for u in range(1, 5):
                    nc.scalar.mul(tmps[u - 1][:, :Tt], x[u], w0s[u])

                # DVE: cross-product AMRs first (only need the DMA'd tiles)
                for k in range(3):
                    a, b = CROSS[k]
                    base = t * 9 + 3 * k
                    nc.vector.affine_mul_reduce(
                        out=dummy[:].broadcast_to((128, Tt)), accum_out=pcol[:, base + 1 : base + 2],
                        in0=x[5 + a], in1=e[b], scale=c2, bias=0.0,
                    )
                    nc.vector.affine_mul_reduce(
                        out=dummy[:].broadcast_to((128, Tt)), accum_out=pcol[:, base + 2 : base + 3],
                        in0=x[5 + b], in1=e[a], scale=-c2, bias=0.0,
                    )

                # DVE: fold the gate terms, then the 3 s*e_k AMRs
                for u in range(4):
                    nc.vector.tensor_add(
                        s_buf[:, :Tt], s_buf[:, :Tt], tmps[u][:, :Tt]
                    )
                for k in range(3):
                    base = t * 9 + 3 * k
                    nc.vector.affine_mul_reduce(
                        out=dummy[:].broadcast_to((128, Tt)), accum_out=pcol[:, base : base + 1],
                        in0=s_buf[:, :Tt], in1=e[k], scale=1.0, bias=0.0,
                    )

            # --- fold partials: acc[:, k] = sum over tiles and terms ---
            pcol3 = pcol[:].rearrange("p (t j) -> p t j", j=9)
            for k in range(3):
                nc.vector.tensor_reduce(
                    out=acc[:, k : k + 1], in_=pcol3[:, :, 3 * k : 3 * k + 3],
                    axis=mybir.AxisListType.XY, op=Alu.add,
                )

            # --- gT = transpose([g|1]): [128, 4] -> [4, 128] via PE ---
            gT_ps = psum.tile([4, 128], f32)
            nc.tensor.transpose(gT_ps[:], acc[:], identity[:])
            gT = state.tile([4, 128], f32)
            nc.scalar.copy(gT[:], gT_ps[:])

            # --- hT = relu(W1b^T(k,m) contracted with gT(k,n)) ---
            h_ps = psum.tile([128, 128], f32)
            nc.tensor.matmul(h_ps[:], lhsT=w1b_s[:], rhs=gT[:], start=True, stop=True)
            hT = state.tile([128, 128], f32)
            nc.scalar.activation(hT[:], h_ps[:], Act.Relu)

            # --- outT = W2^T . hT + b2 ---
            o_ps = psum.tile([3, 128], f32)
            nc.tensor.matmul(o_ps[:], lhsT=w2_s[:], rhs=hT[:], start=True, stop=True)
            oT = state.tile([3, 128], f32)
            nc.scalar.activation(oT[:], o_ps[:], Act.Identity, bias=b2_s[:])
            nc.sync.dma_start(out=outT, in_=oT[:])

    nc.finalize()
    return nc


def _get_nc(w_path0, w_path1):
    key = (
        np.asarray(w_path0, np.float32).tobytes(),
        np.asarray(w_path1, np.float32).tobytes(),
    )
    if _CACHED.get("key") != key:
        _CACHED["nc"] = _build(
            np.asarray(w_path0, np.float32).reshape(5),
            float(np.asarray(w_path1, np.float32).reshape(1)[0]),
        )
        _CACHED["key"] = key
    return _CACHED["nc"]


def _tile_major(shard, C):
    """[BL, N, C] -> [BL, sum_t C*Tt]: per tile, channel-planar planes."""
    blocks = []
    off = 0
    for Tt in TILES:
        blk = shard[:, off : off + Tt, :].transpose(0, 2, 1).reshape(BL, C * Tt)
        blocks.append(blk)
        off += Tt
    return np.ascontiguousarray(np.concatenate(blocks, axis=1))


def _in_maps(feats, edge_attr, W1, b1, W2, b2):
    f32 = np.float32
    W1m = np.ascontiguousarray(W1, f32).reshape(3, 128)
    b1m = np.ascontiguousarray(b1, f32).reshape(1, 128)
    W2m = np.ascontiguousarray(W2, f32).reshape(128, 3)
    b2m = np.ascontiguousarray(b2, f32).reshape(3, 1)
    feats = np.asarray(feats, f32)
    edge_attr = np.asarray(edge_attr, f32)
    maps = []
    for c in range(NCORES):
        sl = slice(c * BL, (c + 1) * BL)
        maps.append(
            {
                "feats": _tile_major(feats[sl], 8),
                "edge": _tile_major(edge_attr[sl], 3),
                "W1": W1m,
                "b1": b1m,
                "W2": W2m,
                "b2": b2m,
            }
        )
    return maps


def run(inputs, trace=False, tmpdir=None):
    """Run on 8 cores; returns (out [B,3], BassKernelResults)."""
    from concourse import bass_utils

    nc = _get_nc(inputs["w_path0"], inputs["w_path1"])
    maps = _in_maps(
        inputs["feats"], inputs["edge_attr"],
        inputs["W1"], inputs["b1"], inputs["W2"], inputs["b2"],
    )
    kw = {}
    if trace:
        kw.update(trace=True, tmpdir=tmpdir)
    res = bass_utils.run_bass_kernel_spmd(
        nc, maps, core_ids=list(range(NCORES)), **kw
    )
    outT_full = np.concatenate([r["outT"] for r in res.results], axis=1)  # [3, B]
    return np.ascontiguousarray(outT_full.T), res


def kernel(feats, edge_attr, w_path0, w_path1, W1, b1, W2, b2):
    out, _ = run(
        dict(
            feats=feats, edge_attr=edge_attr, w_path0=w_path0, w_path1=w_path1,
            W1=W1, b1=b1, W2=W2, b2=b2,
        )
    )
    return out
